# revision 24
# baseline (speedup 1.0000x reference)
"""AdvancedTokenRouter (expert-choice MoE routing) on 8 Trainium2 NeuronCores.

Strategy (data-parallel over tokens, per sharding hint):
  x [8, 4096, 1024] -> one 4096-token shard per core; router W/b replicated.

Launch 1 (heavy, memory-bound): per core, load the 16 MiB x shard, PE-transpose
it in 128x128 blocks (exact fp32), then matmul with the transposed x chunks as
the stationary operand and W.T [128, 64] as the moving operand (4x fewer
moving rows than the logits.T orientation -> measured fastest exact-fp32
variant on HW). Accumulate over 8 K-chunks in PSUM -> logits in natural
token-major layout [4096, 64] -> DMA out.

Host merge (tiny): per-expert k-th largest (k = 614) over all 32768 tokens =
per-expert selection threshold t_e. (A token is in expert e's global top-k iff
logit[n, e] >= t_e -- exact, given distinct values.)

Launch 2 (small, DVE): per core, z = (logits >= t_e) * logits; per-token
best = max_e z; first expert attaining it via reversed-iota-max trick;
selected = best > 0 (thresholds are positive for this distribution; host
falls back to a numpy path otherwise).

Host epilogue: bincount -> load-balance loss (f32 scalar).
"""

import numpy as np

import concourse.bacc as bacc
import concourse.mybir as mybir
import concourse.tile as tile
from concourse import masks
from concourse.bass_utils import run_bass_kernel_spmd

F32 = mybir.dt.float32
I32 = mybir.dt.int32
U8 = mybir.dt.uint8

N_CORES = 8
N_LOC = 4096          # tokens per core
D = 1024
E = 64
KCH = D // 128        # 8 K-chunks
NG = N_LOC // 512     # 8 groups of 512 tokens
N = N_CORES * N_LOC
CAPACITY_FACTOR = 1.2
TOPK = min(max(1, int(N / max(1, E) * CAPACITY_FACTOR)), N)  # 614

_NC_CACHE = {}
LAST_PERF = {}


def _build_launch1(reps=1):
    nc = bacc.Bacc("TRN2", target_bir_lowering=False, debug=False)
    x_d = nc.dram_tensor("x", [N_LOC, D], F32, kind="ExternalInput")
    wt_d = nc.dram_tensor("wt", [D, E], F32, kind="ExternalInput")
    bias_d = nc.dram_tensor("bias", [128, E], F32, kind="ExternalInput")
    lg_d = nc.dram_tensor("logn", [N_LOC, E], F32, kind="ExternalOutput")

    with tile.TileContext(nc) as tc:
        with (
            tc.tile_pool(name="const", bufs=1) as cpool,
            tc.tile_pool(name="xin", bufs=12) as xpool,
            tc.tile_pool(name="xt", bufs=2) as xtpool,
            tc.tile_pool(name="lgs", bufs=2) as lgpool,
            tc.tile_pool(name="tp_ps", bufs=4, space="PSUM") as tppool,
            tc.tile_pool(name="mm_ps", bufs=2, space="PSUM") as mmpool,
        ):
            # W.T as [128, KCH, E]: partition p of chunk k = input dim 128k+p
            wt = cpool.tile([128, KCH, E], F32)
            nc.sync.dma_start(wt[:], wt_d[:].rearrange("(k p) e -> p k e", p=128))
            # bias replicated along free dim is not needed: logits natural
            # layout has experts innermost -> bias is a per-free-element row;
            # broadcast via step-0 outer dims from a [128, E] tile
            bias = cpool.tile([128, E], F32)
            nc.sync.dma_start(bias[:], bias_d[:])
            ident = cpool.tile([128, 128], F32)
            masks.make_identity(nc, ident[:])

            import contextlib
            loop_cm = (tc.For_i(0, reps, 1) if reps > 1
                       else contextlib.nullcontext())
            with loop_cm:
                _emit_launch1_body(nc, tc, x_d, lg_d, wt, bias, ident,
                                   xpool, xtpool, lgpool, tppool, mmpool)
    nc.compile()
    return nc


def _emit_launch1_body(nc, tc, x_d, lg_d, wt, bias, ident,
                       xpool, xtpool, lgpool, tppool, mmpool):
            for g in range(NG):
                xts = []
                for t in range(4):
                    # partition p holds token p*32 + (g*4 + t): keeps the
                    # logits store contiguous (1 KB/partition) and matches
                    # launch2's token layout with no reshuffle
                    xt_in = xpool.tile([128, D], F32, tag="xin")
                    nc.sync.dma_start(xt_in[:], x_d[(g * 4 + t)::32, :])
                    xts.append(xt_in)
                xt = xtpool.tile([128, KCH, 512], F32, tag="xt")
                for k in range(KCH):
                    tp = tppool.tile([128, 512], F32, tag="tp")
                    for t in range(4):
                        nc.tensor.transpose(
                            tp[:, t * 128:(t + 1) * 128],
                            xts[t][:, k * 128:(k + 1) * 128],
                            ident[:])
                    nc.scalar.copy(xt[:, k, :], tp[:])
                mm = mmpool.tile([128, 4, E], F32, tag="mm")
                for t in range(4):
                    for k in range(KCH):
                        nc.tensor.matmul(
                            mm[:, t, :],
                            xt[:, k, t * 128:(t + 1) * 128],
                            wt[:, k, :],
                            start=(k == 0), stop=(k == KCH - 1))
                lg = lgpool.tile([128, 4, E], F32, tag="lg")
                nc.vector.tensor_tensor(
                    lg[:], mm[:],
                    bias[:].unsqueeze(1).to_broadcast([128, 4, E]),
                    op=mybir.AluOpType.add)
                nc.sync.dma_start(
                    lg_d[:].rearrange("(p i) e -> p i e", i=32)
                        [:, g * 4:(g + 1) * 4, :],
                    lg[:])


def _build_launch2(reps=1):
    nc = bacc.Bacc("TRN2", target_bir_lowering=False, debug=False)
    lg_d = nc.dram_tensor("logn", [N_LOC, E], F32, kind="ExternalInput")
    # consts[:, :E] = reversed iota (E - e), consts[:, E:] = thr_e; rows equal
    co_d = nc.dram_tensor("consts", [128, 2 * E], F32, kind="ExternalInput")
    best_d = nc.dram_tensor("best", [128, 32], F32, kind="ExternalOutput")
    asg_d = nc.dram_tensor("asg", [128, 32], I32, kind="ExternalOutput")
    sel_d = nc.dram_tensor("sel", [128, 32], U8, kind="ExternalOutput")

    with tile.TileContext(nc) as tc:
        with (
            tc.tile_pool(name="const", bufs=1) as cpool,
            tc.tile_pool(name="work", bufs=1) as wpool,
            tc.tile_pool(name="big", bufs=1) as bigpool,
        ):
            co = cpool.tile([128, 2 * E], F32)
            nc.sync.dma_start(co[:], co_d[:])

            import contextlib
            loop_cm = (tc.For_i(0, reps, 1) if reps > 1
                       else contextlib.nullcontext())
            with loop_cm:
                _emit_launch2_body(nc, tc, lg_d, best_d, asg_d, sel_d, co,
                                   wpool, bigpool)
    nc.compile()
    return nc


def _emit_launch2_body(nc, tc, lg_d, best_d, asg_d, sel_d, co, wpool, bigpool):
            # token q*32+i lands on partition q, tile i (contiguous rows/part)
            ln = bigpool.tile([128, 32, E], F32)
            nc.sync.dma_start(
                ln[:], lg_d[:].rearrange("(q i) e -> q i e", i=32))

            ge = bigpool.tile([128, 32, E], F32)
            nc.vector.tensor_tensor(
                ge[:], ln[:],
                co[:, E:2 * E].unsqueeze(1).to_broadcast([128, 32, E]),
                op=mybir.AluOpType.is_ge)
            z = bigpool.tile([128, 32, E], F32)
            nc.vector.tensor_tensor(z[:], ge[:], ln[:],
                                    op=mybir.AluOpType.mult)

            best = wpool.tile([128, 32], F32)
            nc.vector.reduce_max(best[:], z[:], axis=mybir.AxisListType.X)
            eq = bigpool.tile([128, 32, E], F32)
            nc.vector.tensor_tensor(
                eq[:], z[:], best[:].unsqueeze(2).to_broadcast([128, 32, E]),
                op=mybir.AluOpType.is_equal)
            cand = bigpool.tile([128, 32, E], F32)
            nc.vector.tensor_tensor(
                cand[:], eq[:],
                co[:, 0:E].unsqueeze(1).to_broadcast([128, 32, E]),
                op=mybir.AluOpType.mult)
            mxc = wpool.tile([128, 32], F32)
            nc.vector.reduce_max(mxc[:], cand[:], axis=mybir.AxisListType.X)

            sel = wpool.tile([128, 32], F32)
            nc.vector.tensor_scalar(sel[:], best[:], 0.0, None,
                                    op0=mybir.AluOpType.is_gt)
            # assigned = sel * (65 - mxc) - 1   (mxc = 64 - argmin_e)
            t1 = wpool.tile([128, 32], F32)
            nc.vector.tensor_scalar(t1[:], mxc[:], -1.0, 65.0,
                                    op0=mybir.AluOpType.mult,
                                    op1=mybir.AluOpType.add)
            t2 = wpool.tile([128, 32], F32)
            nc.vector.tensor_tensor(t2[:], t1[:], sel[:],
                                    op=mybir.AluOpType.mult)
            asg_f = wpool.tile([128, 32], F32)
            nc.vector.tensor_scalar(asg_f[:], t2[:], -1.0, None,
                                    op0=mybir.AluOpType.add)
            asg_i = wpool.tile([128, 32], I32)
            nc.vector.tensor_copy(asg_i[:], asg_f[:])
            sel_u = wpool.tile([128, 32], U8)
            nc.scalar.copy(sel_u[:], sel[:])

            nc.sync.dma_start(best_d[:], best[:])
            nc.sync.dma_start(asg_d[:], asg_i[:])
            nc.sync.dma_start(sel_d[:], sel_u[:])


def _host_fallback(logN, k):
    """Full-precision host implementation of the selection stage, used only
    if a threshold is non-positive (never for the target distribution)."""
    logT = logN.T
    Etot, Ntot = logT.shape
    order = np.argsort(-logT, axis=1, kind="stable")[:, :k]
    best = np.full(Ntot, -np.inf, np.float32)
    for e in range(Etot):
        np.maximum.at(best, order[e], logT[e, order[e]])
    tok_exp = np.full(Ntot, Etot, np.int32)
    for e in range(Etot - 1, -1, -1):
        idx = order[e]
        hit = logT[e, idx] == best[idx]
        tok_exp[idx[hit]] = e
    selected = tok_exp < Etot
    assigned = np.where(selected, tok_exp, -1).astype(np.int32)
    best_out = np.where(selected, best, 0.0).astype(np.float32)
    return best_out, assigned, selected


def kernel(x, W, b, detach_inputs=0):
    x = np.asarray(x, dtype=np.float32)
    W = np.asarray(W, dtype=np.float32)
    b = np.asarray(b, dtype=np.float32)
    bsz, seqlen, d = x.shape
    assert (bsz, seqlen, d) == (N_CORES, N_LOC, D) and W.shape == (E, D)

    if "l1" not in _NC_CACHE:
        _NC_CACHE["l1"] = _build_launch1()
    if "l2" not in _NC_CACHE:
        _NC_CACHE["l2"] = _build_launch2()

    wt = np.ascontiguousarray(W.T)
    bias = np.ascontiguousarray(np.broadcast_to(b, (128, E)).astype(np.float32))
    in1 = [{"x": np.ascontiguousarray(x[c]), "wt": wt, "bias": bias}
           for c in range(N_CORES)]
    r1 = run_bass_kernel_spmd(_NC_CACHE["l1"], in1, core_ids=list(range(N_CORES)))
    LAST_PERF["l1_ns"] = r1.exec_time_ns
    logN = np.concatenate([r["logn"] for r in r1.results], axis=0)  # [N, E]

    kth = np.partition(logN, N - TOPK, axis=0)[N - TOPK]  # k-th largest, [E]
    thr = kth.astype(np.float32)

    if not (thr > 0).all():
        best, assigned, selected = _host_fallback(logN, TOPK)
    else:
        co_row = np.concatenate(
            [float(E) - np.arange(E, dtype=np.float32), thr])
        co = np.ascontiguousarray(np.broadcast_to(co_row, (128, 2 * E)))
        in2 = [{"logn": np.ascontiguousarray(logN[c * N_LOC:(c + 1) * N_LOC]),
                "consts": co} for c in range(N_CORES)]
        r2 = run_bass_kernel_spmd(_NC_CACHE["l2"], in2,
                                  core_ids=list(range(N_CORES)))
        LAST_PERF["l2_ns"] = r2.exec_time_ns
        best = np.concatenate([r["best"].reshape(-1) for r in r2.results])
        assigned = np.concatenate([r["asg"].reshape(-1) for r in r2.results])
        selected = np.concatenate(
            [r["sel"].reshape(-1) for r in r2.results]).astype(bool)

    counts = np.bincount(assigned[selected], minlength=E).astype(np.float32)
    mean_load = np.float32(counts.mean())
    loss = np.float32(
        np.mean((counts - mean_load) ** 2, dtype=np.float32)
        / (mean_load + np.float32(1e-9)))
    return (best.astype(np.float32), assigned.astype(np.int32),
            selected, loss)
